# revision 53
# baseline (speedup 1.0000x reference)
"""Trainium2 Bass kernel for nn_CrossAttentionLayer.

Reference computation (per batch element b):
    q = x @ Wq            [N, INNER]   (heads: INNER = H*Dh)
    k = ctx @ Wk          [J, INNER]
    v = ctx @ Wv          [J, INNER]
    sim = q_h @ k_h.T * scale   per head -> softmax over J -> @ v_h
    out = concat_heads @ Wo + bo

Sharding: batch (B=8) across 8 cores, one batch element per core, weights
replicated.  No collectives needed.

Per-core structure (bf16 matmul operands, fp32 PSUM):
  - x loaded bf16 straight to SBUF (gpsimd casting DMA, 2 chunks), then
    PE-transposed into xT (8 transposes packed per psum slot, one packed
    bf16 eviction each); ctxT via DMA transpose; weights staged bf16 in
    DRAM, with gpsimd dummy reads sequencing the shared DMA device
  - KT [INNER, J] = Wk.T @ ctxT   (stationary Wk chunks, moving ctxT)
  - V  [J, INNER] = ctxT.T @ Wv, stored per head as 64 cols + a ones col
  - QT [INNER, N] = Wq.T @ xT
  - attention in 2 n-halves of 1024; heads software-pipelined against the
    exp stream on ACT (PV lags S by one head); QT chunks interleave into
    half 0 and the half-0 out-projection interleaves into half 1:
      S^T[j, n] = KT_h.T @ QT_h  (K=64)          -> psum [128, 1024]
      P^T = exp(scale * S^T)  (ACT)              -> pts bf16
      per n-tile: O[n, 0:65] = sum_jc pts_chunk.T @ Vpad_h
        (65-col moving operand; col 64 = softmax denominator)
      normalize per head: rden = 1/den (DVE), onat = O * rden (bf16)
      PE-transpose onat pair [128,128] -> ot chunk [INNER, N] layout
  - out = OT.T @ Wo + bias (DVE add), DMA out per [128, 512] tile; the
    last head's half-1 PV/transpose is fused per n-tile with the final
    out-projection tiles to shrink the tail
"""

import sys

if "/opt/trn_rl_repo" not in sys.path:
    sys.path.insert(0, "/opt/trn_rl_repo")

import numpy as np

import concourse.bass as bass
import concourse.mybir as mybir
import concourse.bacc as bacc
import concourse.tile as tile
from concourse import bass_utils
from concourse.masks import make_identity
from concourse.tile_rust import add_dep_helper

P = 128
B, N, J = 8, 2048, 1024
QD, CD, H, Dh = 1024, 768, 16, 64
INNER = H * Dh
NT = N // P      # 16 n tiles
JC = J // P      # 8 context chunks
QC = QD // P     # 8 x-feature chunks
CC = CD // P     # 6 ctx-feature chunks
IC = INNER // P  # 8 inner chunks
NBW = 512        # moving-operand block width
NH_N = N // 2                # 1024 n per attention half
NTH = NH_N // P              # 8 n tiles per half
XG = 2                       # x staging chunks
XT_G = NT // XG              # 8 n-tiles per x staging chunk
SCALE = float(Dh) ** -0.5

F32 = mybir.dt.float32
BF16 = mybir.dt.bfloat16
EXP = mybir.ActivationFunctionType.Exp
MULT = mybir.AluOpType.mult
ADD = mybir.AluOpType.add

_CACHE = {}


def _build_module():
    nc = bacc.Bacc("TRN2", target_bir_lowering=False, debug=False)

    x_d = nc.dram_tensor("x", [N, QD], F32, kind="ExternalInput")
    ctx_d = nc.dram_tensor("context", [J, CD], F32, kind="ExternalInput")
    wq_d = nc.dram_tensor("Wq", [QD, INNER], F32, kind="ExternalInput")
    wk_d = nc.dram_tensor("Wk", [CD, INNER], F32, kind="ExternalInput")
    wv_d = nc.dram_tensor("Wv", [CD, INNER], F32, kind="ExternalInput")
    wo_d = nc.dram_tensor("Wo", [INNER, QD], F32, kind="ExternalInput")
    bo_d = nc.dram_tensor("bo", [QD], F32, kind="ExternalInput")
    out_d = nc.dram_tensor("out", [N, QD], F32, kind="ExternalOutput")

    with tile.TileContext(nc) as tc:
        _emit(nc, tc, x_d, ctx_d, wq_d, wk_d, wv_d, wo_d, bo_d, out_d)

    nc.compile()
    return nc


def _emit(nc, tc, x_d, ctx_d, wq_d, wk_d, wv_d, wo_d, bo_d, out_d):
    from contextlib import ExitStack

    est = ExitStack()
    with est:
        # ---------- constants ----------
        const = est.enter_context(tc.tile_pool(name="const", bufs=1))
        ident = const.tile([P, P], BF16, name="ident")
        make_identity(nc, ident[:])
        ones_bf = const.tile([1, P], BF16, name="ones_bf")
        nc.vector.memset(ones_bf[:], 1.0)
        dummy = const.tile([1, 4], F32, name="dummy")

        # ---------- bf16 casts staged in DRAM (flat = 1 descriptor each) ----
        dram = est.enter_context(tc.tile_pool(name="dram", bufs=1, space="DRAM"))
        ctx_bf = dram.tile([J, CD], BF16, name="ctx_bf")
        wq_bf = dram.tile([QD, INNER], BF16, name="wq_bf")
        wk_bf = dram.tile([CD, INNER], BF16, name="wk_bf")
        wv_bf = dram.tile([CD, INNER], BF16, name="wv_bf")
        wo_bf = dram.tile([INNER, QD], BF16, name="wo_bf")

        # ---------- persistent activations / inputs ----------
        qkv = est.enter_context(tc.tile_pool(name="qkv", bufs=1))
        qt = [qkv.tile([P, N], BF16, name=f"qt{c}", tag=f"qt{c}") for c in range(IC)]
        kt = [qkv.tile([P, J], BF16, name=f"kt{c}", tag=f"kt{c}") for c in range(IC)]
        # v padded: per head 64 cols of V then a ones column (65 per head)
        vp = [qkv.tile([P, H * 65], BF16, name=f"vp{c}", tag=f"vp{c}")
              for c in range(JC)]
        wq_pool = est.enter_context(tc.tile_pool(name="wq_pool", bufs=1))
        wq_sb = wq_pool.tile([P, QC * INNER], BF16, name="wq_sb")
        xT_p = est.enter_context(tc.tile_pool(name="xT_p", bufs=1))
        xT = [xT_p.tile([P, N], BF16, name=f"xT{c}", tag=f"xT{c}")
              for c in range(QC)]

        # ---------- psum pools: 2x[128,1024] (4 banks) + 4x[128,512] ------
        spsum = est.enter_context(
            tc.tile_pool(name="spsum", bufs=2, space="PSUM"))
        aux = est.enter_context(
            tc.tile_pool(name="aux", bufs=4, space="PSUM"))

        # ---------- attention pts ring (before vctx: stack order) ---------
        pts_p = est.enter_context(tc.tile_pool(name="pts_p", bufs=2))
        onat_p = est.enter_context(tc.tile_pool(name="onat_p", bufs=2))
        rd_p = est.enter_context(tc.tile_pool(name="rd_p", bufs=4))

        # ---------- transient ctx/x-path pools (LIFO: xn, kctx, vctx) -----
        vctx = ExitStack()
        wv_pool = vctx.enter_context(tc.tile_pool(name="wv_pool", bufs=1))
        wv_sb = wv_pool.tile([P, CC * INNER], BF16, name="wv_sb")
        ctxT_p = vctx.enter_context(tc.tile_pool(name="ctxT_p", bufs=1))
        ctxT = [ctxT_p.tile([P, J], BF16, name=f"ctxT{c}", tag=f"ctxT{c}")
                for c in range(CC)]
        kctx = ExitStack()
        wk_pool = kctx.enter_context(tc.tile_pool(name="wk_pool", bufs=1))
        wk_sb = wk_pool.tile([P, CC * INNER], BF16, name="wk_sb")
        xn_p = ExitStack()
        xn_pool = xn_p.enter_context(tc.tile_pool(name="xn_pool", bufs=1))

        # ---------- DMA issue order ----------
        # Pool (gpsimd) carries the casts.  add_dep_helper pins each later
        # cast behind the earlier dependent load so the scheduler cannot
        # hoist it onto the shared DMA device early.
        def x_nat_dma(g):
            # two flat sub-DMAs per chunk (fewer, smaller ring entries)
            xn = xn_pool.tile([P, XT_G * QD], BF16, name=f"x_nat{g}", tag="xn")
            half_rows = XT_G * P // 2
            for s in range(2):
                r0 = g * XT_G * P + s * half_rows
                nc.gpsimd.dma_start(
                    xn[:, s * (XT_G // 2) * QD:(s + 1) * (XT_G // 2) * QD]
                    .rearrange("p (t c) -> p t c", c=QD),
                    x_d[r0:r0 + half_rows].rearrange("(t p) c -> p t c", p=P))
            return xn

        nc.gpsimd.dma_start(ctx_bf[:].flatten(), ctx_d[:].flatten())
        wk_cast = nc.gpsimd.dma_start(wk_bf[:].flatten(), wk_d[:].flatten())
        x_nat = [x_nat_dma(0), None]
        for cc in range(CC):
            ctxT_last = nc.sync.dma_start(
                ctxT[cc][:], ctx_bf[:, cc * P:(cc + 1) * P], transpose=True)
        wk_load = nc.sync.dma_start(
            wk_sb[:].rearrange("p (c n) -> p c n", c=CC),
            wk_bf[:].rearrange("(c p) n -> p c n", p=P))

        def aux_tile():
            return aux.tile([P, NBW], F32, name="aux", tag="aux")

        # ---------- emit helpers ----------
        def emit_xt(g):
            # PE-transpose x_nat chunk g -> xT[:, g*1024:(g+1)*1024]
            for qc in range(QC):
                sp = spsum.tile([P, NH_N], F32, name="sp", tag="sp")
                spb = sp[:].bitcast(BF16)
                for k in range(XT_G):
                    nc.tensor.matmul(
                        spb[:, k * P:(k + 1) * P],
                        x_nat[g][:, k * QD + qc * P: k * QD + (qc + 1) * P],
                        ident[:], is_transpose=True, start=True, stop=True)
                nc.vector.tensor_copy(
                    xT[qc][:, g * XT_G * P:(g + 1) * XT_G * P],
                    spb[:, 0:XT_G * P])

        def emit_kt(ic):
            for jb in range(J // NBW):
                kp = aux_tile()
                for cc in range(CC):
                    nc.tensor.matmul(
                        kp[:],
                        wk_sb[:, cc * INNER + ic * P: cc * INNER + (ic + 1) * P],
                        ctxT[cc][:, jb * NBW:(jb + 1) * NBW],
                        start=(cc == 0), stop=(cc == CC - 1),
                    )
                nc.vector.tensor_copy(kt[ic][:, jb * NBW:(jb + 1) * NBW], kp[:])

        def emit_v(vb, jc, after=None):
            vpp = aux_tile()
            for cc in range(CC):
                mm = nc.tensor.matmul(
                    vpp[:],
                    ctxT[cc][:, jc * P:(jc + 1) * P],
                    wv_sb[:, cc * INNER + vb * NBW: cc * INNER + (vb + 1) * NBW],
                    start=(cc == 0), stop=(cc == CC - 1),
                )
                if after is not None and cc == 0:
                    add_dep_helper(mm.ins, after.ins, False, "pe order")
            hpb = NBW // Dh  # heads per block = 8
            dst = vp[jc][:, vb * hpb * 65:(vb + 1) * hpb * 65]
            dst = dst.rearrange("p (h e) -> p h e", e=65)[:, :, 0:64]
            src = vpp[:].rearrange("p (h e) -> p h e", e=Dh)
            nc.vector.tensor_copy(dst, src)
            if vb == 0:
                ones_cols = vp[jc][:].rearrange(
                    "p (h e) -> p h e", e=65)[:, :, 64:65]
                nc.vector.memset(ones_cols, 1.0)

        def emit_qt(ic, nb_lo=0, nb_hi=N // NBW):
            for nb in range(nb_lo, nb_hi):
                qp = aux_tile()
                for qc in range(QC):
                    nc.tensor.matmul(
                        qp[:],
                        wq_sb[:, qc * INNER + ic * P: qc * INNER + (ic + 1) * P],
                        xT[qc][:, nb * NBW:(nb + 1) * NBW],
                        start=(qc == 0), stop=(qc == QC - 1),
                    )
                nc.vector.tensor_copy(qt[ic][:, nb * NBW:(nb + 1) * NBW], qp[:])

        last_s_mm = [None]

        def emit_s_pair(h, half, k, tiles):
            """S^T + exp for jc in [2k, 2k+2); appends pts tiles."""
            ic, po = h // 2, (h % 2) * Dh
            n0 = half * NH_N
            for jc in range(2 * k, 2 * k + 2):
                sp = spsum.tile([P, NH_N], F32, name="sp", tag="sp")
                for sub in range(NH_N // NBW):
                    last_s_mm[0] = nc.tensor.matmul(
                        sp[:, sub * NBW:(sub + 1) * NBW],
                        kt[ic][po:po + Dh, jc * P:(jc + 1) * P],
                        qt[ic][po:po + Dh,
                               n0 + sub * NBW: n0 + (sub + 1) * NBW],
                        start=True, stop=True,
                    )
                ptile = pts_p.tile([P, NH_N], BF16, name=f"pts{jc}",
                                   tag=f"pts{jc}")
                nc.scalar.activation(ptile[:], sp[:], EXP, scale=SCALE)
                tiles.append(ptile)

        def emit_s(h, half):
            tiles = []
            for k in range(4):
                emit_s_pair(h, half, k, tiles)
            return tiles

        def emit_pv_ntl(h, pts, onat_tile, ntl):
            hh = h % 2
            pv = aux_tile()
            for jc in range(JC):
                nc.tensor.matmul(
                    pv[:, 0:65],
                    pts[jc][:, ntl * P:(ntl + 1) * P],
                    vp[jc][:, h * 65: h * 65 + 65],
                    start=(jc == 0), stop=(jc == JC - 1),
                )
            rden = rd_p.tile([P, 1], F32, name="rden", tag="rden")
            nc.vector.reciprocal(rden[:], pv[:, 64:65])
            nc.vector.tensor_scalar(
                onat_tile[:, hh * Dh:(hh + 1) * Dh],
                pv[:, 0:Dh], rden[:], None, MULT)

        def emit_pv(h, pts, onat_tiles):
            """O-natural PV + per-head normalize into onat pair tiles."""
            for ntl in range(NTH):
                emit_pv_ntl(h, pts, onat_tiles[ntl], ntl)

        def emit_tr_ntl(ic, half, onat_tile, ntl):
            n0 = half * NH_N
            tp = aux_tile()
            tpb = tp[:].bitcast(BF16)[:, 0:P]
            nc.tensor.matmul(tpb, onat_tile[:], ident[:],
                             is_transpose=True, start=True, stop=True)
            nc.vector.tensor_copy(
                ot[ic][:, n0 + ntl * P: n0 + (ntl + 1) * P], tpb)

        def emit_transpose(ic, half, onat_tiles):
            for ntl in range(NTH):
                emit_tr_ntl(ic, half, onat_tiles[ntl], ntl)

        def emit_outproj(nt, qb):
            op = aux_tile()
            for ic in range(IC):
                nc.tensor.matmul(
                    op[:],
                    ot[ic][:, nt * P:(nt + 1) * P],
                    wo_sb[:, ic * QD + qb * NBW: ic * QD + (qb + 1) * NBW],
                    start=(ic == 0), stop=(ic == IC - 1),
                )
            ostage = ostage_p.tile([P, NBW], F32, name="ostage", tag="ostage")
            nc.vector.tensor_tensor(
                ostage[:], op[:], bias_bc[:, qb * NBW:(qb + 1) * NBW], op=ADD)
            nc.sync.dma_start(
                out_d[nt * P:(nt + 1) * P, qb * NBW:(qb + 1) * NBW], ostage[:])

        def emit_bias():
            for qb in range(QD // NBW):
                bp = aux_tile()
                nc.tensor.matmul(
                    bp[:], ones_bf[:, :], bo_bf[:, qb * NBW:(qb + 1) * NBW],
                    start=True, stop=True)
                nc.vector.tensor_copy(bias_bc[:, qb * NBW:(qb + 1) * NBW], bp[:])

        # ---------- schedule: phase A (x transposes + KT) ----------
        emit_xt(0)
        x_nat[1] = x_nat_dma(1)   # WAR on chunk-0 transposes sequences it
        wq_cast = nc.gpsimd.dma_start(wq_bf[:].flatten(), wq_d[:].flatten())
        add_dep_helper(wq_cast.ins, wk_load.ins, True, "dma order")
        add_dep_helper(wq_cast.ins, ctxT_last.ins, True, "dma order")
        wq_load = nc.sync.dma_start(
            wq_sb[:].rearrange("p (c n) -> p c n", c=QC),
            wq_bf[:].rearrange("(c p) n -> p c n", p=P))
        wv_cast = nc.gpsimd.dma_start(wv_bf[:].flatten(), wv_d[:].flatten())
        add_dep_helper(wv_cast.ins, wq_load.ins, True, "dma order")
        wv_load = nc.sync.dma_start(
            wv_sb[:].rearrange("p (c n) -> p c n", c=CC),
            wv_bf[:].rearrange("(c p) n -> p c n", p=P))
        wo_cast = nc.gpsimd.dma_start(wo_bf[:].flatten(), wo_d[:].flatten())
        add_dep_helper(wo_cast.ins, wv_load.ins, True, "dma order")
        for ic in range(4):
            emit_kt(ic)
        emit_xt(1)
        for ic in range(4, IC):
            emit_kt(ic)
        xn_p.close()
        kctx.close()   # wk SBUF free

        onat_of = {}

        def onat_pair(ic, half):
            if (ic, half) not in onat_of:
                onat_of[(ic, half)] = [
                    onat_p.tile([P, P], BF16, name=f"on{t}", tag=f"on{t}")
                    for t in range(NTH)]
            return onat_of[(ic, half)]

        def u_s(h, half):
            pts_of[(h, half)] = emit_s(h, half)

        def u_pv(h, half):
            emit_pv(h, pts_of.pop((h, half)), onat_pair(h // 2, half))

        def u_tr(ic, half):
            emit_transpose(ic, half, onat_of.pop((ic, half)))

        # half 0 start: S(0,0)/S(1,0), V vb0 (PV of heads 0-7 needs only
        # vb0), with borrowed half-1 units of heads 0/1 keeping the exp
        # stream busy through the V block; V vb1 before any PV of heads 8+.
        pts_of = {}
        emit_qt(0)
        u_s(0, 0)
        u_s(1, 0)
        for jc in range(JC):
            emit_v(0, jc)
        u_pv(0, 0)
        u_s(0, 1)
        u_pv(1, 0)
        for jc in range(JC):
            emit_v(1, jc)
        vctx.close()   # frees wv/ctxT SBUF for ot/wo

        # ---------- late pools: attention output (transposed) + Wo --------
        o_bf = est.enter_context(tc.tile_pool(name="o_bf", bufs=1))
        # ot[ic] [P, N]: normalized attention output, transposed layout
        # [INNER, N]; head h lives in chunk h//2, partitions (h%2)*64.
        ot = [o_bf.tile([P, N], BF16, name=f"ot{c}", tag=f"ot{c}")
              for c in range(IC)]
        wo_pool = est.enter_context(tc.tile_pool(name="wo_pool", bufs=1))
        wo_sb = wo_pool.tile([P, IC * QD], BF16, name="wo_sb")
        late = est.enter_context(tc.tile_pool(name="late", bufs=1))
        bo_bf = late.tile([1, QD], BF16, name="bo_bf")
        bias_bc = late.tile([P, QD], BF16, name="bias_bc")
        ostage_p = est.enter_context(tc.tile_pool(name="ostage_p", bufs=2))

        nc.sync.dma_start(
            wo_sb[:].rearrange("p (c n) -> p c n", c=IC),
            wo_bf[:].rearrange("(c p) n -> p c n", p=P))
        nc.gpsimd.dma_start(bo_bf[:], bo_d[:].unsqueeze(0))

        u_s(1, 1)
        u_pv(0, 1)
        emit_bias()
        u_tr(0, 0)
        emit_qt(1, 0, 2)
        u_s(2, 0)
        u_pv(1, 1)
        u_tr(0, 1)
        emit_qt(1, 2, 4)
        u_s(3, 0)
        emit_qt(2, 0, 2)
        u_pv(2, 0)
        emit_qt(2, 2, 4)

        # half 0 steady state: PV(h-1) ntl-pairs and QT pieces interleave
        # into S(h)'s jc-pairs (the exp stream paces S; PE fills between).
        for h in range(4, H):
            qi = h // 2 + 1
            nb0 = (h % 2) * 2
            hp = h - 1
            prev = pts_of.pop((hp, 0))
            on_prev = onat_pair(hp // 2, 0)
            tiles = []
            emit_s_pair(h, 0, 0, tiles)
            emit_pv_ntl(hp, prev, on_prev[0], 0)
            emit_pv_ntl(hp, prev, on_prev[1], 1)
            emit_s_pair(h, 0, 1, tiles)
            emit_pv_ntl(hp, prev, on_prev[2], 2)
            emit_pv_ntl(hp, prev, on_prev[3], 3)
            if qi < IC:
                emit_qt(qi, nb0, nb0 + 1)
            emit_s_pair(h, 0, 2, tiles)
            emit_pv_ntl(hp, prev, on_prev[4], 4)
            emit_pv_ntl(hp, prev, on_prev[5], 5)
            emit_s_pair(h, 0, 3, tiles)
            emit_pv_ntl(hp, prev, on_prev[6], 6)
            emit_pv_ntl(hp, prev, on_prev[7], 7)
            if qi < IC:
                emit_qt(qi, nb0 + 1, nb0 + 2)
            pts_of[(h, 0)] = tiles
            if hp % 2 == 1:
                u_tr(hp // 2, 0)
        u_pv(H - 1, 0)
        u_tr((H - 1) // 2, 0)

        # half 1 (heads 0/1 already done): half-0 out-projection interleaves
        op_queue = [(nt, qb) for nt in range(NTH) for qb in range(2)]
        u_s(2, 1)
        for h in range(3, H):
            hp = h - 1
            prev = pts_of.pop((hp, 1))
            on_prev = onat_pair(hp // 2, 1)
            tiles = []
            emit_s_pair(h, 1, 0, tiles)
            emit_pv_ntl(hp, prev, on_prev[0], 0)
            emit_pv_ntl(hp, prev, on_prev[1], 1)
            emit_s_pair(h, 1, 1, tiles)
            emit_pv_ntl(hp, prev, on_prev[2], 2)
            emit_pv_ntl(hp, prev, on_prev[3], 3)
            emit_outproj(*op_queue.pop(0))
            emit_s_pair(h, 1, 2, tiles)
            emit_pv_ntl(hp, prev, on_prev[4], 4)
            emit_pv_ntl(hp, prev, on_prev[5], 5)
            emit_s_pair(h, 1, 3, tiles)
            emit_pv_ntl(hp, prev, on_prev[6], 6)
            emit_pv_ntl(hp, prev, on_prev[7], 7)
            pts_of[(h, 1)] = tiles
            if hp % 2 == 1:
                u_tr(hp // 2, 1)
            if h >= H - 3:
                emit_outproj(*op_queue.pop(0))

        # fused tail: last head's PV+transpose feed half-1 outproj per tile
        h = H - 1
        pts_last = pts_of.pop((h, 1))
        on_last = onat_pair(h // 2, 1)
        for ntl in range(NTH):
            emit_pv_ntl(h, pts_last, on_last[ntl], ntl)
            emit_tr_ntl(h // 2, 1, on_last[ntl], ntl)
            emit_outproj(NTH + ntl, 0)
            emit_outproj(NTH + ntl, 1)
        assert not op_queue, op_queue


def _get_module():
    if "nc" not in _CACHE:
        _CACHE["nc"] = _build_module()
    return _CACHE["nc"]


def kernel(x, context, Wq, Wk, Wv, Wo, bo):
    nc = _get_module()
    x = np.asarray(x, dtype=np.float32)
    context = np.asarray(context, dtype=np.float32)
    Wq = np.asarray(Wq, dtype=np.float32)
    Wk = np.asarray(Wk, dtype=np.float32)
    Wv = np.asarray(Wv, dtype=np.float32)
    Wo = np.asarray(Wo, dtype=np.float32)
    bo = np.asarray(bo, dtype=np.float32)

    in_maps = [
        {
            "x": np.ascontiguousarray(x[b]),
            "context": np.ascontiguousarray(context[b]),
            "Wq": Wq, "Wk": Wk, "Wv": Wv, "Wo": Wo, "bo": bo,
        }
        for b in range(B)
    ]
    res = bass_utils.run_bass_kernel_spmd(nc, in_maps, core_ids=list(range(B)))
    return np.stack([res.results[b]["out"] for b in range(B)], axis=0)


if __name__ == "__main__":
    nc = _get_module()
    print("module built and compiled OK")
